# revision 3
# baseline (speedup 1.0000x reference)
"""DCNv3 x2 + proj gating: fused single-launch SPMD kernel for 8 trn2 cores.

Sharding: batch n = core//4, token-row quarter q = core%4 (16 of 64 rows).
Channel-major (NCHW) layouts for all matmuls; token-major for LN/softmax/
sampling-weight pipeline; static 5x5 patch sampling with offsets clamped to
(-1, 1); AllGather halo exchange of attn1 edge rows between blocks.
"""
import numpy as np

import concourse.bacc as bacc
import concourse.mybir as mybir
from concourse.tile import TileContext

f32 = mybir.dt.float32
fp16 = mybir.dt.float16
ALU = mybir.AluOpType
ACT = mybir.ActivationFunctionType

P = 128
G = 8            # groups
K = 9            # points
NC = 8           # cores
H = W = 64
RQ = 16          # token rows per core
TOKQ = RQ * W    # 1024 tokens per core
GR, GC = 20, 68  # vpad grid rows/cols (vp rows r0..r0+20, vp cols 0..68)
XGR, XGC = 20, 66  # x grid rows/cols (img rows r0-1..r0+19, img cols -1..65)
NCH = 8          # token chunks of 128
CH256 = 4        # chunks of 256 tokens (4 img rows)
CLAMP = 0.999999
f16 = np.float16


def build_nc(debug=False):
    nc = bacc.Bacc("TRN2", target_bir_lowering=False)

    # ---------------- external inputs (per-core data, same shapes) -----
    ei = lambda n, s, dt=fp16: nc.dram_tensor(n, s, dt, kind="ExternalInput")
    xg = [ei(f"xg{t}", [P, XGR * XGC]) for t in range(2)]        # x window grids
    xq = [ei(f"xq{t}", [P, TOKQ], f32) for t in range(2)]        # x quarter fp32
    vmask = ei("vmask", [P, GR * 64])                            # v validity
    gmask = ei("gmask", [P, XGR * XGC])                          # attn1 grid validity
    selmask = ei("selmask", [P, 4 * 4], f32)                     # halo src select [slot, src]
    ident = ei("ident", [P, P])
    # weights (shared content across cores): per block
    WBLK = []
    for b in ("a", "b"):
        d = {}
        d["in_w"] = ei(f"{b}_in_w", [P, 2 * 2 * P])       # [cin_t][cout_t] blocks
        d["out_w"] = ei(f"{b}_out_w", [P, 2 * 2 * P])
        d["ox_w"] = ei(f"{b}_ox_w", [P, 2 * 72])          # [cin_t] -> 72
        d["oy_w"] = ei(f"{b}_oy_w", [P, 2 * 72])
        d["mk_w"] = ei(f"{b}_mk_w", [P, 2 * 72])
        d["dwdiag"] = ei(f"{b}_dwdiag", [P, 2 * 9 * P])   # [ct][tap] diag mats
        d["in_b"] = ei(f"{b}_in_b", [P, 2], f32)          # per ctile col
        d["out_b"] = ei(f"{b}_out_b", [P, 2], f32)
        d["dw_b"] = ei(f"{b}_dw_b", [P, 2], f32)
        d["ox_b"] = ei(f"{b}_ox_b", [P, 1], f32)          # [72 rows used]
        d["oy_b"] = ei(f"{b}_oy_b", [P, 1], f32)
        d["mk_b"] = ei(f"{b}_mk_b", [P, 1], f32)
        d["ln_g"] = ei(f"{b}_ln_g", [P, 256], f32)        # replicated across parts
        d["ln_b"] = ei(f"{b}_ln_b", [P, 256], f32)
        WBLK.append(d)
    proj_w = ei("proj_w", [P, 2 * 2 * P])
    proj_b = ei("proj_b", [P, 2], f32)

    out_d = nc.dram_tensor("out", [2 * P, TOKQ], f32, kind="ExternalOutput")
    dbg = {}
    if debug:
        for nm, sh, dt in (("d_vg", [P, 2 * GR * GC], fp16),
                           ("d_f", [P, 2 * TOKQ], fp16),
                           ("d_ox", [P, NCH * 72], f32),
                           ("d_m", [P, NCH * 72], f32),
                           ("d_A", [P, NCH * 200], f32),
                           ("d_agg", [P, 2 * TOKQ], fp16),
                           ("d_y1", [P, 2 * TOKQ], fp16),
                           ("d_xgb", [P, 2 * XGR * XGC], fp16)):
            dbg[nm] = nc.dram_tensor(nm, sh, dt, kind="ExternalOutput")

    with TileContext(nc) as tc:
        with (
            tc.tile_pool(name="const", bufs=1) as cpool,
            tc.tile_pool(name="work", bufs=1) as pool,
            tc.tile_pool(name="wrp", bufs=2) as wpool,
            tc.tile_pool(name="ps", bufs=4, space="PSUM") as psp,
            tc.tile_pool(name="pst", bufs=2, space="PSUM") as pst,
            tc.tile_pool(name="dram", bufs=1, space="DRAM") as dpool,
        ):
            # ---- load constants ----
            L = lambda d, nm: cpool.tile_from(d[:, :], name=nm)
            xg_t = [L(xg[t], f"xg{t}") for t in range(2)]
            xq_t = [L(xq[t], f"xq{t}") for t in range(2)]
            vmask_t = L(vmask, "vmask")
            gmask_t = L(gmask, "gmask")
            selmask_t = L(selmask, "selmask")
            ident_t = L(ident, "ident")
            wt = []
            for b in range(2):
                d = {k: L(v, f"w{b}_{k}") for k, v in WBLK[b].items()}
                wt.append(d)
            proj_w_t = L(proj_w, "proj_w")
            proj_b_t = L(proj_b, "proj_b")

            adump = dpool.tile([2 * 100, NCH * P], fp16)       # A^T dram bounce
            ag_in = dpool.tile([2 * P, 4 * 64], fp16)          # halo contribution
            ag_out = dpool.tile([4 * 2 * P, 4 * 64], fp16)     # gathered (group of 4)

            def block(blk, xgrids, first):
                """One DCNv3 block. xgrids: [2] SBUF grid tiles [P, XGR*XGC].
                Returns y1 (attn1^T) [2] tiles [P, TOKQ] fp16."""
                wd = wt[blk]
                tag = f"b{blk}"

                # ===== input_proj -> vgrid (padded value image) ==========
                vgrid = [pool.tile([P, GR * GC], fp16, name=f"{tag}vg{t}", tag=f"vg{t}")
                         for t in range(2)]
                for ct in range(2):
                    nc.vector.memset(vgrid[ct][:, :], 0.0)
                vc = [pool.tile([P, GR * 64], fp16, name=f"{tag}vc{t}", tag=["halo", "hm"][t])
                      for t in range(2)]
                for co in range(2):      # output ctile
                    for r0, rn in ((0, 8), (8, 8), (16, 4)):
                        ps = psp.tile([P, 512], f32, tag="mmps")
                        for ci in range(2):
                            mov = xgrids[ci][:, :].rearrange(
                                "p (r c) -> p r c", r=XGR)[:, r0:r0 + rn, 1:65]
                            nc.tensor.matmul(
                                ps[:, :rn * 64],
                                wd["in_w"][:, (ci * 2 + co) * P:(ci * 2 + co + 1) * P],
                                mov, start=(ci == 0), stop=(ci == 1))
                        nc.scalar.activation(
                            vc[co][:, r0 * 64:(r0 + rn) * 64], ps[:, :rn * 64],
                            ACT.Identity, bias=wd["in_b"][:, co:co + 1])
                for ct in range(2):
                    nc.vector.tensor_tensor(
                        vc[ct][:, :], vc[ct][:, :], vmask_t[:, :], ALU.mult)
                    dst = vgrid[ct][:, :].rearrange(
                        "p (r c) -> p r c", r=GR)[:, :, 2:66]
                    src = vc[ct][:, :].rearrange("p (r c) -> p r c", r=GR)
                    nc.vector.tensor_copy(dst, src)

                # ===== dwconv (9 diag matmuls) -> y^T =====================
                y = [pool.tile([P, TOKQ], fp16, name=f"{tag}y{t}", tag=["s01", "s23"][t])
                     for t in range(2)]
                for ct in range(2):
                    for r0 in (0, 8):
                        ps = psp.tile([P, 512], f32, tag="mmps")
                        for tap in range(9):
                            dy, dx = tap // 3 - 1, tap % 3 - 1
                            mov = xgrids[ct][:, :].rearrange(
                                "p (r c) -> p r c", r=XGR)[
                                :, 2 + dy + r0:2 + dy + r0 + 8, 1 + dx:1 + dx + 64]
                            nc.tensor.matmul(
                                ps[:, :],
                                wd["dwdiag"][:, (ct * 9 + tap) * P:(ct * 9 + tap + 1) * P],
                                mov, start=(tap == 0), stop=(tap == 8))
                        nc.scalar.activation(
                            y[ct][:, r0 * 64:(r0 + 8) * 64], ps[:, :],
                            ACT.Identity, bias=wd["dw_b"][:, ct:ct + 1])

                # ===== transpose y -> token-major, LN + gelu ==============
                ytok = pool.tile([P, NCH * 256], f32, name=f"{tag}ytok", tag="ytok")
                for ch in range(NCH):
                    for ct in range(2):
                        tps = pst.tile([P, P], fp16, tag="trps")
                        nc.tensor.transpose(
                            tps[:, :], y[ct][:, ch * P:(ch + 1) * P],
                            ident_t[:, :])
                        nc.vector.tensor_copy(
                            ytok[:, ch * 256 + ct * P:ch * 256 + (ct + 1) * P],
                            tps[:, :])
                y3 = ytok[:, :].rearrange("p (c f) -> p c f", c=NCH)
                mu = pool.tile([P, NCH], f32, name=f"{tag}mu", tag="mu")
                nc.vector.tensor_reduce(mu[:, :], y3, axis=mybir.AxisListType.X,
                                        op=ALU.add)
                nc.vector.tensor_scalar(mu[:, :], mu[:, :], 1.0 / 256, None,
                                        ALU.mult)
                ysq = pool.tile([P, NCH * 256], f32, name=f"{tag}ysq", tag="ysq")
                nc.scalar.activation(ysq[:, :], ytok[:, :], ACT.Square)
                var = pool.tile([P, NCH], f32, name=f"{tag}var", tag="var")
                nc.vector.tensor_reduce(
                    var[:, :], ysq[:, :].rearrange("p (c f) -> p c f", c=NCH),
                    axis=mybir.AxisListType.X, op=ALU.add)
                nc.vector.tensor_scalar(var[:, :], var[:, :], 1.0 / 256, None,
                                        ALU.mult)
                musq = pool.tile([P, NCH], f32, name=f"{tag}musq", tag="musq")
                nc.scalar.activation(musq[:, :], mu[:, :], ACT.Square)
                nc.vector.tensor_tensor(var[:, :], var[:, :], musq[:, :],
                                        ALU.subtract)
                nc.vector.tensor_scalar(var[:, :], var[:, :], 1e-5, None,
                                        ALU.add)
                std = pool.tile([P, NCH], f32, name=f"{tag}std", tag="std")
                nc.scalar.activation(std[:, :], var[:, :], ACT.Sqrt)
                rstd = pool.tile([P, NCH], f32, name=f"{tag}rstd", tag="rstd")
                nc.vector.reciprocal(rstd[:, :], std[:, :])
                mub = mu[:, :].unsqueeze(2).broadcast_to((P, NCH, 256))
                rstdb = rstd[:, :].unsqueeze(2).broadcast_to((P, NCH, 256))
                nc.vector.tensor_tensor(y3, y3, mub, ALU.subtract)
                nc.vector.tensor_tensor(y3, y3, rstdb, ALU.mult)
                lng = wd["ln_g"][:, :].unsqueeze(1).broadcast_to((P, NCH, 256))
                lnb = wd["ln_b"][:, :].unsqueeze(1).broadcast_to((P, NCH, 256))
                nc.vector.tensor_tensor(y3, y3, lng, ALU.mult)
                nc.vector.tensor_tensor(y3, y3, lnb, ALU.add)
                ftok = pool.tile([P, NCH * 256], fp16, name=f"{tag}ftok", tag="ftok")
                nc.scalar.activation(ftok[:, :], ytok[:, :], ACT.Gelu)
                # transpose back -> f^T channel-major
                f = [pool.tile([P, TOKQ], fp16, name=f"{tag}f{t}", tag=f"f{t}")
                     for t in range(2)]
                for ch in range(NCH):
                    for ct in range(2):
                        tps = pst.tile([P, P], fp16, tag="trps")
                        nc.tensor.transpose(
                            tps[:, :],
                            ftok[:, ch * 256 + ct * P:ch * 256 + (ct + 1) * P],
                            ident_t[:, :])
                        nc.vector.tensor_copy(
                            f[ct][:, ch * P:(ch + 1) * P], tps[:, :])

                # ===== ox / oy / mk projections (72 rows each) ============
                omt = {}
                for nm in ("ox", "oy", "mk"):
                    omT = pool.tile([P, TOKQ], fp16, name=f"{tag}{nm}T", tag=f"omT{nm}")
                    for r0 in (0, 512):
                        ps = psp.tile([P, 512], f32, tag="mmps")
                        for ci in range(2):
                            nc.tensor.matmul(
                                ps[:72, :],
                                wd[nm + "_w"][:, ci * 72:(ci + 1) * 72],
                                f[ci][:, r0:r0 + 512],
                                start=(ci == 0), stop=(ci == 1))
                        nc.scalar.activation(
                            omT[:72, r0:r0 + 512], ps[:72, :],
                            ACT.Identity, bias=wd[nm + "_b"][:72, 0:1])
                    omt[nm] = omT
                # transpose to token-major [128, (ch, 72)]
                tok = {}
                for nm in ("ox", "oy", "mk"):
                    t = pool.tile([P, NCH * 72], f32, name=f"{tag}{nm}tok",
                                  tag={"ox": "ytok2", "oy": "ysq2", "mk": "mk"}[nm])
                    for ch in range(NCH):
                        tps = pst.tile([P, 72], fp16, tag="trps")
                        nc.tensor.transpose(
                            tps[:, :], omt[nm][:72, ch * P:(ch + 1) * P],
                            ident_t[:72, :72])
                        nc.vector.tensor_copy(
                            t[:, ch * 72:(ch + 1) * 72], tps[:, :])
                    tok[nm] = t

                # ===== softmax over k (tok-major) =========================
                m = pool.tile([P, NCH * 72], f32, name=f"{tag}m", tag="m")
                nc.scalar.activation(m[:, :], tok["mk"][:, :], ACT.Exp)
                z = pool.tile([P, NCH * 8], f32, name=f"{tag}z", tag="z")
                nc.vector.tensor_reduce(
                    z[:, :], m[:, :].rearrange("p (c g k) -> p (c g) k", c=NCH, g=G),
                    axis=mybir.AxisListType.X, op=ALU.add)
                rz = pool.tile([P, NCH * 8], f32, name=f"{tag}rz", tag="rz")
                nc.vector.reciprocal(rz[:, :], z[:, :])
                rzb = rz[:, :].unsqueeze(2).broadcast_to((P, NCH * 8, K))
                m3 = m[:, :].rearrange("p (a k) -> p a k", k=K)
                nc.vector.tensor_tensor(m3, m3, rzb, ALU.mult)

                # ===== clamp offsets, hat coefficients ====================
                for nm in ("ox", "oy"):
                    nc.vector.tensor_scalar(tok[nm][:, :], tok[nm][:, :],
                                            CLAMP, None, ALU.min)
                    nc.vector.tensor_scalar(tok[nm][:, :], tok[nm][:, :],
                                            -CLAMP, None, ALU.max)
                hat = {}
                for nm in ("ox", "oy"):
                    h = pool.tile([P, NCH * 72 * 3], fp16, name=f"{tag}h{nm}", tag=f"h{nm}")
                    h3 = h[:, :].rearrange("p (a s) -> p a s", s=3)
                    t1 = pool.tile([P, NCH * 72], f32, name=f"{tag}t1", tag="hats")
                    for s in range(3):
                        nc.vector.tensor_scalar(
                            t1[:, :], tok[nm][:, :], -1.0, float(s),
                            ALU.mult, op1=ALU.add)
                        nc.vector.tensor_scalar(
                            h3[:, :, s], tok[nm][:, :], float(s - 2), None,
                            ALU.subtract)
                        nc.vector.tensor_tensor(h3[:, :, s], h3[:, :, s],
                                                t1[:, :], ALU.min)
                    nc.scalar.activation(h[:, :], h[:, :], ACT.Relu)
                    hat[nm] = h3
                # cym = cy * m
                cym = pool.tile([P, NCH * 72 * 3], fp16, name=f"{tag}cym", tag="cym")
                cym3 = cym[:, :].rearrange("p (a s) -> p a s", s=3)
                mb = m[:, :].unsqueeze(2).broadcast_to((P, NCH * 72, 3))
                nc.vector.tensor_tensor(cym3, hat["oy"], mb, ALU.mult)

                # ===== A5 products + reduce over k -> A ===================
                # process 4 token-chunks at a time to halve SBUF footprint
                A = pool.tile([P, NCH * 200], f32, name=f"{tag}A", tag="A")
                A5 = pool.tile([P, 4 * G * 25 * K], fp16, name=f"{tag}A5",
                               tag="A5")
                cymv = cym[:, :].rearrange("p (c g k s) -> p c g k s",
                                           c=NCH, g=G, k=K)
                cxv = hat["ox"].rearrange("p (c g k) s -> p c g k s",
                                          c=NCH, g=G)
                for hf in range(2):
                    ch0 = hf * 4
                    nc.vector.memset(A5[:, :], 0.0)
                    A5v = A5[:, :].rearrange("p (c g t k) -> p c g t k",
                                             c=4, g=G, t=25)
                    for k in range(K):
                        kx, ky = k // 3 - 1, k % 3 - 1
                        for j in range(3):
                            tap0 = (ky + 1 + j) * 5 + (kx + 1)
                            dst = A5v[:, :, :, tap0:tap0 + 3, k]
                            a = cymv[:, ch0:ch0 + 4, :, k, j].unsqueeze(
                                3).broadcast_to((P, 4, G, 3))
                            b = cxv[:, ch0:ch0 + 4, :, k, :]
                            nc.vector.tensor_tensor(dst, a, b, ALU.mult)
                    nc.vector.tensor_reduce(
                        A[:, ch0 * 200:(ch0 + 4) * 200],
                        A5[:, :].rearrange("p (c gt k) -> p c gt k",
                                           c=4, k=K),
                        axis=mybir.AxisListType.X, op=ALU.add)
                A16 = pool.tile([P, NCH * 200], fp16, name=f"{tag}A16", tag="A16")
                nc.vector.tensor_copy(A16[:, :], A[:, :])

                # ===== A -> DRAM (via PE transpose) =======================
                a16t = pool.tile([100, NCH * P], fp16, name=f"{tag}a16t", tag="a16t")
                for half in range(2):
                    for ch in range(NCH):
                        tps = pst.tile([100, P], fp16, tag="trps")
                        nc.tensor.transpose(
                            tps[:, :],
                            A16[:, ch * 200 + half * 100:ch * 200 + (half + 1) * 100],
                            ident_t[:, :])
                        nc.vector.tensor_copy(
                            a16t[:, ch * P:(ch + 1) * P], tps[:, :])
                    nc.sync.dma_start(
                        adump[half * 100:(half + 1) * 100, :], a16t[:, :])

                # ===== sampling aggregation ===============================
                agg16 = [pool.tile([P, TOKQ], fp16, name=f"{tag}ag16{t}",
                                   tag=f"ag16{t}") for t in range(2)]
                for ct in range(2):
                    for c4 in range(CH256):
                        wrep = wpool.tile([P, 25 * 256], fp16, tag="wrep")
                        for g in range(4):
                            src = adump[:, :].rearrange("q (f) -> q f")[
                                ct * 100 + g * 25:ct * 100 + (g + 1) * 25,
                                c4 * 256:(c4 + 1) * 256]
                            nc.sync.dma_start(
                                wrep[g * 32:(g + 1) * 32, :].rearrange(
                                    "p (t f) -> p t f", t=25),
                                src.unsqueeze(0).broadcast_to((32, 25, 256)))
                        Pt = pool.tile([P, 25 * 256], fp16, name=f"{tag}P",
                                       tag="Pt")
                        for r in range(4):
                            row = c4 * 4 + r
                            src_im = vgrid[ct][:, :].rearrange(
                                "p (a b) -> p a b", a=GR)[
                                :, row:row + 5, :]
                            # iterate (ty, tx, col): addr ty*68 + tx + col
                            im = src_im.rearrange("p a b -> p (a b)")
                            imv = mybir_view(im, [(68, 5), (1, 5), (1, 64)])
                            pv = mybir_view(
                                Pt[:, :], [(1280, 5), (256, 5), (1, 64)],
                                offset=r * 64)
                            wv = mybir_view(
                                wrep[:, :], [(1280, 5), (256, 5), (1, 64)],
                                offset=r * 64)
                            nc.vector.tensor_tensor(pv, imv, wv, ALU.mult)
                        # tree-reduce 25 taps
                        p5 = Pt[:, :].rearrange("p (ty f) -> p ty f", ty=5)
                        s01 = pool.tile([P, 1280], fp16, name=f"{tag}s01", tag="s01b")
                        s23 = pool.tile([P, 1280], fp16, name=f"{tag}s23", tag="s23b")
                        nc.vector.tensor_tensor(s01[:, :], p5[:, 0, :],
                                                p5[:, 1, :], ALU.add)
                        nc.vector.tensor_tensor(s23[:, :], p5[:, 2, :],
                                                p5[:, 3, :], ALU.add)
                        nc.vector.tensor_tensor(s01[:, :], s01[:, :],
                                                s23[:, :], ALU.add)
                        nc.vector.tensor_tensor(s01[:, :], s01[:, :],
                                                p5[:, 4, :], ALU.add)
                        # now sum 5 tx: s01 [P, (tx 5, 256)]
                        q5 = s01[:, :].rearrange("p (tx f) -> p tx f", tx=5)
                        t01 = pool.tile([P, 256], fp16, name=f"{tag}t01",
                                        tag="t01")
                        t23 = pool.tile([P, 256], fp16, name=f"{tag}t23",
                                        tag="t23")
                        nc.vector.tensor_tensor(t01[:, :], q5[:, 0, :],
                                                q5[:, 1, :], ALU.add)
                        nc.vector.tensor_tensor(t23[:, :], q5[:, 2, :],
                                                q5[:, 3, :], ALU.add)
                        nc.vector.tensor_tensor(t01[:, :], t01[:, :],
                                                t23[:, :], ALU.add)
                        nc.vector.tensor_tensor(
                            agg16[ct][:, c4 * 256:(c4 + 1) * 256],
                            t01[:, :], q5[:, 4, :], ALU.add)

                # ===== output projection =================================
                y1 = [pool.tile([P, TOKQ], fp16, name=f"{tag}y1_{t}", tag=f"{tag}y1_{t}")
                      for t in range(2)]
                for co in range(2):
                    for r0 in (0, 512):
                        ps = psp.tile([P, 512], f32, tag="mmps")
                        for ci in range(2):
                            nc.tensor.matmul(
                                ps[:, :],
                                wd["out_w"][:, (ci * 2 + co) * P:(ci * 2 + co + 1) * P],
                                agg16[ci][:, r0:r0 + 512],
                                start=(ci == 0), stop=(ci == 1))
                        nc.scalar.activation(
                            y1[co][:, r0:r0 + 512], ps[:, :],
                            ACT.Identity, bias=wd["out_b"][:, co:co + 1])
                if debug and first:
                    nc.sync.dma_start(dbg["d_vg"][:, :GR * GC], vgrid[0][:, :])
                    nc.sync.dma_start(dbg["d_vg"][:, GR * GC:], vgrid[1][:, :])
                    nc.sync.dma_start(dbg["d_f"][:, :TOKQ], f[0][:, :])
                    nc.sync.dma_start(dbg["d_f"][:, TOKQ:], f[1][:, :])
                    nc.sync.dma_start(dbg["d_ox"][:, :], tok["ox"][:, :])
                    nc.sync.dma_start(dbg["d_m"][:, :], m[:, :])
                    nc.sync.dma_start(dbg["d_A"][:, :], A[:, :])
                    nc.sync.dma_start(dbg["d_agg"][:, :TOKQ], agg16[0][:, :])
                    nc.sync.dma_start(dbg["d_agg"][:, TOKQ:], agg16[1][:, :])
                    nc.sync.dma_start(dbg["d_y1"][:, :TOKQ], y1[0][:, :])
                    nc.sync.dma_start(dbg["d_y1"][:, TOKQ:], y1[1][:, :])
                return y1

            # ================== block a ==================================
            y1 = block(0, xg_t, False)

            # ================== halo exchange ============================
            for ct in range(2):
                yv = y1[ct][:, :].rearrange("p (r c) -> p r c", r=RQ)
                nc.sync.dma_start(
                    ag_in[ct * P:(ct + 1) * P, 0:128], yv[:, 14:16, :])
                nc.sync.dma_start(
                    ag_in[ct * P:(ct + 1) * P, 128:256], yv[:, 0:2, :])
            nc.gpsimd.collective_compute(
                "AllGather", ALU.bypass,
                replica_groups=[[0, 1, 2, 3], [4, 5, 6, 7]],
                ins=[ag_in[:, :].opt()],
                outs=[ag_out[:, :].opt()])

            # assemble xgrid_b = attn1 image window
            xgb = [pool.tile([P, XGR * XGC], fp16, name=f"xgb{t}")
                   for t in range(2)]
            halo = pool.tile([P, 4 * 4 * 64], fp16, name="halo")
            for ct in range(2):
                nc.vector.memset(xgb[ct][:, :], 0.0)
                # local rows -> grid rows 1..17, cols 1..65
                dst = xgb[ct][:, :].rearrange("p (r c) -> p r c", r=XGR)[
                    :, 2:18, 1:65]
                nc.vector.tensor_copy(
                    dst, y1[ct][:, :].rearrange("p (r c) -> p r c", r=RQ))
                # halo: bring all 4 sources' 4 slots, select via mask
                for src in range(4):
                    nc.sync.dma_start(
                        halo[:, src * 256:(src + 1) * 256],
                        ag_out[src * 2 * P + ct * P:src * 2 * P + (ct + 1) * P, :])
                hv = halo[:, :].rearrange("p (s t c) -> p s t c", s=4, t=4)
                sel = selmask_t[:, :].rearrange("p (t s) -> p t s", t=4)
                hm = pool.tile([P, 4 * 4 * 64], f32, name="hm")
                hmv = hm[:, :].rearrange("p (t s c) -> p t s c", t=4, s=4)
                nc.vector.tensor_tensor(
                    hmv, hv.transpose([0, 2, 1, 3]),
                    sel.unsqueeze(3).broadcast_to((P, 4, 4, 64)), ALU.mult)
                hsel = pool.tile([P, 4 * 64], f32, name="hsel")
                nc.vector.tensor_reduce(
                    hsel[:, :],
                    hm[:, :].rearrange("p (t s c) -> p t c s", t=4, s=4),
                    axis=mybir.AxisListType.X, op=ALU.add)
                hselv = hsel[:, :].rearrange("p (t c) -> p t c", t=4)
                xgbv = xgb[ct][:, :].rearrange("p (r c) -> p r c", r=XGR)
                nc.vector.tensor_copy(xgbv[:, 0:2, 1:65], hselv[:, 0:2, :])
                nc.vector.tensor_copy(xgbv[:, 18:20, 1:65], hselv[:, 2:4, :])
                nc.vector.tensor_tensor(xgb[ct][:, :], xgb[ct][:, :],
                                        gmask_t[:, :], ALU.mult)
            if debug:
                nc.sync.dma_start(dbg["d_xgb"][:, :XGR * XGC], xgb[0][:, :])
                nc.sync.dma_start(dbg["d_xgb"][:, XGR * XGC:], xgb[1][:, :])

            # ================== block b ==================================
            y2 = block(1, xgb, True)

            # ================== final proj + gating ======================
            for co in range(2):
                for r0 in (0, 512):
                    ps = psp.tile([P, 512], f32, tag="mmps")
                    for ci in range(2):
                        nc.tensor.matmul(
                            ps[:, :],
                            proj_w_t[:, (ci * 2 + co) * P:(ci * 2 + co + 1) * P],
                            y2[ci][:, r0:r0 + 512],
                            start=(ci == 0), stop=(ci == 1))
                    at = pool.tile([P, 512], f32, name="attn", tag="attn")
                    nc.scalar.activation(at[:, :], ps[:, :], ACT.Identity,
                                         bias=proj_b_t[:, co:co + 1])
                    nc.vector.tensor_tensor(at[:, :], at[:, :],
                                            xq_t[co][:, r0:r0 + 512], ALU.mult)
                    nc.sync.dma_start(out_d[co * P:(co + 1) * P, r0:r0 + 512],
                                      at[:, :])
    nc.compile()
    return nc


def mybir_view(ap, dims, offset=0):
    """Build a strided free-dim view [(step, num), ...] on a 2D AP."""
    import concourse.bass as bass
    base = ap
    # use rearrange-free manual AP construction via successive ops:
    # simplest: use the AP dataclass directly
    new_ap = list(base.ap)
    part = new_ap[0]
    free = [[int(s), int(n)] for s, n in dims]
    return type(base)(
        tensor=base.tensor,
        offset=base.offset + offset,
        ap=[list(part)] + free,
        const_val=None,
        runtime_checks=base.runtime_checks,
        dep_tracking_offset=None,
    )


def prep_shared(inp):
    """Core-independent tensors."""
    d = {}

    def wblocks(Wm):  # [256, 256] -> [128, (ci, co, 128)]
        a = np.zeros((P, 4 * P), np.float32)
        for ci in range(2):
            for co in range(2):
                a[:, (ci * 2 + co) * P:(ci * 2 + co + 1) * P] = \
                    Wm[ci * P:(ci + 1) * P, co * P:(co + 1) * P]
        return a.astype(f16)

    for b in ("a", "b"):
        d[f"{b}_in_w"] = wblocks(inp[f"{b}_in_w"])
        d[f"{b}_out_w"] = wblocks(inp[f"{b}_out_w"])
        off_w = inp[f"{b}_off_w"]   # [256, 144]
        mk_w = inp[f"{b}_mk_w"]     # [256, 72]
        oxw = np.zeros((P, 2 * 72), np.float32)
        oyw = np.zeros((P, 2 * 72), np.float32)
        mkw = np.zeros((P, 2 * 72), np.float32)
        for ci in range(2):
            oxw[:, ci * 72:(ci + 1) * 72] = off_w[ci * P:(ci + 1) * P, 0::2]
            oyw[:, ci * 72:(ci + 1) * 72] = off_w[ci * P:(ci + 1) * P, 1::2]
            mkw[:, ci * 72:(ci + 1) * 72] = mk_w[ci * P:(ci + 1) * P, :]
        d[f"{b}_ox_w"] = oxw.astype(f16)
        d[f"{b}_oy_w"] = oyw.astype(f16)
        d[f"{b}_mk_w"] = mkw.astype(f16)
        dw_w = inp[f"{b}_dw_w"]     # [3, 3, 1, 256]
        dg = np.zeros((P, 2 * 9 * P), np.float32)
        for ct in range(2):
            for tap in range(9):
                dy, dx = tap // 3, tap % 3
                blk = dg[:, (ct * 9 + tap) * P:(ct * 9 + tap + 1) * P]
                blk[np.arange(P), np.arange(P)] = dw_w[dy, dx, 0,
                                                       ct * P:(ct + 1) * P]
        d[f"{b}_dwdiag"] = dg.astype(f16)
        for nm, src in (("in_b", inp[f"{b}_in_b"]), ("out_b", inp[f"{b}_out_b"]),
                        ("dw_b", inp[f"{b}_dw_b"])):
            a = np.zeros((P, 2), np.float32)
            a[:, 0] = src[:P]
            a[:, 1] = src[P:]
            d[f"{b}_{nm}"] = a
        off_b = inp[f"{b}_off_b"]
        oxb = np.zeros((P, 1), np.float32)
        oyb = np.zeros((P, 1), np.float32)
        oxb[:72, 0] = off_b[0::2]
        oyb[:72, 0] = off_b[1::2]
        d[f"{b}_ox_b"] = oxb
        d[f"{b}_oy_b"] = oyb
        mkb = np.zeros((P, 1), np.float32)
        mkb[:72, 0] = inp[f"{b}_mk_b"]
        d[f"{b}_mk_b"] = mkb
        d[f"{b}_ln_g"] = np.tile(inp[f"{b}_ln_g"][None, :], (P, 1)).astype(np.float32)
        d[f"{b}_ln_b"] = np.tile(inp[f"{b}_ln_b"][None, :], (P, 1)).astype(np.float32)
    d["proj_w"] = wblocks(inp["proj_w"])
    pb = np.zeros((P, 2), np.float32)
    pb[:, 0] = inp["proj_b"][:P]
    pb[:, 1] = inp["proj_b"][P:]
    d["proj_b"] = pb
    d["ident"] = np.eye(P, dtype=f16)
    return d


def prep_core(inp, shared, core):
    n, q = core // 4, core % 4
    r0 = RQ * q
    x = inp["x"][n]  # [256, 64, 64]
    d = dict(shared)
    for ct in range(2):
        xgrid = np.zeros((P, XGR, XGC), np.float32)
        for i in range(XGR):
            r = r0 - 2 + i
            if 0 <= r < H:
                xgrid[:, i, 1:65] = x[ct * P:(ct + 1) * P, r, :]
        d[f"xg{ct}"] = xgrid.reshape(P, -1).astype(f16)
        d[f"xq{ct}"] = np.ascontiguousarray(
            x[ct * P:(ct + 1) * P, r0:r0 + RQ, :].reshape(P, TOKQ)).astype(np.float32)
    vm = np.zeros((P, GR, 64), np.float32)
    for i in range(GR):
        if 1 <= r0 - 1 + i <= 64:
            vm[:, i, :] = 1.0
    d["vmask"] = vm.reshape(P, -1).astype(f16)
    gm = np.zeros((P, XGR, XGC), np.float32)
    for i in range(XGR):
        if 0 <= r0 - 2 + i < H:
            gm[:, i, 1:65] = 1.0
    d["gmask"] = gm.reshape(P, -1).astype(f16)
    sel = np.zeros((4, 4), np.float32)
    if q >= 1:
        sel[0, q - 1] = 1.0
        sel[1, q - 1] = 1.0
    if q <= 2:
        sel[2, q + 1] = 1.0
        sel[3, q + 1] = 1.0
    d["selmask"] = np.tile(sel.reshape(1, 16), (P, 1)).astype(np.float32)
    return d


def run(inputs, debug=False, trace=False):
    import concourse.bass_utils as bu
    if trace:
        from trn_agent_boot.trn_boot import _ntff_profile_via_ctypes
        hook = _ntff_profile_via_ctypes('/opt/axon/libaxon_pjrt.so')
        mod = types.ModuleType('antenv.axon_hooks')
        mod.get_axon_ntff_profile_hook = lambda: hook
        mod.set_axon_ntff_profile_hook = lambda h: None
        sys.modules['antenv.axon_hooks'] = mod
        bu.upload_artifacts = lambda dd: "local://" + dd
    nc = build_nc(debug=debug)
    shared = prep_shared(inputs)
    in_maps = [prep_core(inputs, shared, c) for c in range(NC)]
    res = bu.run_bass_kernel_spmd(nc, in_maps, core_ids=list(range(NC)),
                                  trace=trace)
    out = np.zeros((2, 256, H, W), np.float32)
    for c in range(NC):
        n, q = c // 4, c % 4
        r0 = RQ * q
        o = res.results[c]["out"]  # [256, 1024]
        out[n, :, r0:r0 + RQ, :] = o.reshape(256, RQ, W)
    return out, res




TRACE = False
LAST_EXEC_NS = None
_CACHE = {}


def _traced_utils():
    import sys, types
    import concourse.bass_utils as bu
    try:
        from trn_agent_boot.trn_boot import _ntff_profile_via_ctypes
        hook = _ntff_profile_via_ctypes('/opt/axon/libaxon_pjrt.so')
        if hook is not None and 'antenv.axon_hooks' not in sys.modules:
            mod = types.ModuleType('antenv.axon_hooks')
            mod.get_axon_ntff_profile_hook = lambda: hook
            mod.set_axon_ntff_profile_hook = lambda h: None
            sys.modules['antenv.axon_hooks'] = mod
        bu.upload_artifacts = lambda dd: "local://" + dd
        return bu, True
    except Exception:
        return bu, False


def kernel(**inputs):
    """Full DCNv3x2+proj gating on 8 NeuronCores. inputs as setup_inputs()."""
    global LAST_EXEC_NS
    inputs = {k: np.asarray(v) for k, v in inputs.items()}
    import concourse.bass_utils as bu
    trace_ok = False
    if TRACE:
        bu, trace_ok = _traced_utils()
    if "nc" not in _CACHE:
        _CACHE["nc"] = build_nc(debug=False)
    nc = _CACHE["nc"]
    shared = prep_shared(inputs)
    in_maps = [prep_core(inputs, shared, c) for c in range(NC)]
    try:
        res = bu.run_bass_kernel_spmd(nc, in_maps, core_ids=list(range(NC)),
                                      trace=TRACE and trace_ok)
    except Exception:
        res = bu.run_bass_kernel_spmd(nc, in_maps, core_ids=list(range(NC)))
    if getattr(res, "exec_time_ns", None):
        LAST_EXEC_NS = res.exec_time_ns
    out = np.zeros((2, 256, H, W), np.float32)
    for c in range(NC):
        n, q = c // 4, c % 4
        r0 = RQ * q
        out[n, :, r0:r0 + RQ, :] = res.results[c]["out"].reshape(256, RQ, W)
    return np.ascontiguousarray(out)
